# revision 36
# baseline (speedup 1.0000x reference)
"""Trainium2 Bass kernel for nn_ABS_MHAtt (masked two-round multi-head attention).

Strategy: pure data-parallel over batch (B=16 -> 2 batches per NeuronCore, 8 cores,
no collectives). Host-side preprocessing (inside kernel()) pre-transposes
activations/weights into the [contraction, free] layouts the TensorEngine wants and
pre-converts everything to bf16, so the device kernel does zero layout conversion.

Per-core device kernel (per batch):
  - qhT/khT projections in transposed form [o, i]; v projected in natural form [j, o]
    directly into an "augmented" layout with a ones column per head (the ones column
    makes the PV/AV matmul also produce the softmax row-sum).
  - Per head: scores computed transposed [j, i] (contraction over d=64, head pairs
    row-tiled onto the two PE array halves), exp on ScalarE, masking by multiplying
    with (1-mask)^T (split across VectorE and GpSimdE), PV/AV with E as the
    stationary operand, one fused broadcast-multiply normalize per head, and DMA
    xbar transposes (not TensorE) to repack [i, d] tiles back to [d, i].

v2 scheduling: the two batches are software-pipelined at the core level — batch 1's
projection matmul groups are interleaved into batch 0's attention phase, and batch
0's output projection into batch 1's attention phase, so the PE never idles long
enough for the HAM clock gate to re-throttle. Input DMAs ride the ScalarE HWDGE
queue; weights, transposes and output stores ride the Sync queue, keeping the
latency-critical xbar transposes out of the bulk-load FIFO.
"""

import os
import sys

import numpy as np


def _ensure_concourse():
    try:
        import concourse.bass  # noqa: F401
        return
    except Exception:
        pass
    for p in ("/opt/trn_rl_repo", "/root/.axon_site/_ro/trn_rl_repo"):
        if os.path.isdir(p) and p not in sys.path:
            sys.path.insert(0, p)
            try:
                import concourse.bass  # noqa: F401
                return
            except Exception:
                sys.path.remove(p)
    raise ImportError("cannot import concourse (bass)")


B, L, HS = 16, 512, 1024
H, D = 16, 64
NCORES = 8
BPC = B // NCORES  # batches per core
SCALE = 1.0 / 8.0  # 1/sqrt(D)
AUGW = 65  # per-head augmented width (D + ones column)

_CACHE = {}


def _build_nc():
    _ensure_concourse()
    import concourse.bass as bass  # noqa: F401
    import concourse.mybir as mybir
    import concourse.tile as tile
    from concourse import bacc
    from contextlib import ExitStack

    bf = mybir.dt.bfloat16
    f32 = mybir.dt.float32
    Exp = mybir.ActivationFunctionType.Exp

    nc = bacc.Bacc()

    # all inputs host-preswizzled to [128, free] per-partition-contiguous
    # layouts so every load is one cheap 2D DMA
    qt = nc.declare_dram_parameter("qt", [BPC, 128, 8 * L], bf, isOutput=False)
    kt = nc.declare_dram_parameter("kt", [BPC, 128, 8 * L], bf, isOutput=False)
    vt = nc.declare_dram_parameter("vt", [BPC, 128, 8 * L], bf, isOutput=False)
    imt = nc.declare_dram_parameter("imt", [BPC, 128, 8 * L], bf, isOutput=False)
    aug = nc.declare_dram_parameter(
        "aug", [BPC, 128, 4 * H * AUGW], bf, isOutput=False
    )
    kp1 = nc.declare_dram_parameter("kp1", [BPC, 128, 4 * L], bf, isOutput=False)
    kp2 = nc.declare_dram_parameter("kp2", [BPC, 128, 4 * L], bf, isOutput=False)
    wq = nc.declare_dram_parameter("wq", [128, 8 * HS], bf, isOutput=False)
    wk = nc.declare_dram_parameter("wk", [128, 8 * HS], bf, isOutput=False)
    wv = nc.declare_dram_parameter("wv", [128, 8 * HS], bf, isOutput=False)
    wm = nc.declare_dram_parameter("wm", [128, 8 * HS], bf, isOutput=False)
    idt = nc.declare_dram_parameter("idt", [128, 128], bf, isOutput=False)
    out = nc.declare_dram_parameter("out", [BPC, L, HS], bf, isOutput=True)

    with ExitStack() as ctx:
        tc = ctx.enter_context(tile.TileContext(nc))
        consts = ctx.enter_context(tc.tile_pool(name="consts", bufs=1))
        inp = ctx.enter_context(tc.tile_pool(name="inp", bufs=1))
        proj = ctx.enter_context(tc.tile_pool(name="proj", bufs=2))
        ework = ctx.enter_context(tc.tile_pool(name="ework", bufs=2))
        small = ctx.enter_context(tc.tile_pool(name="small", bufs=3))
        evac = ctx.enter_context(tc.tile_pool(name="evac", bufs=2))
        psA = ctx.enter_context(tc.tile_pool(name="psA", bufs=3, space="PSUM"))
        psT = ctx.enter_context(tc.tile_pool(name="psT", bufs=2, space="PSUM"))

        w_sb = {}

        def load_weight_half(name, wext, half):
            if half == 0 and name not in w_sb:
                t = consts.tile([128, 8, HS], bf, tag=name, name=name + "_sb")
                w_sb[name] = t
            t = w_sb[name]
            nc.sync.dma_start(
                out=t[:, half * 4 : (half + 1) * 4, :],
                in_=wext[:, half * 4 * HS : (half + 1) * 4 * HS],
            )

        def load_x_half(t, ext, b, half):
            nc.sync.dma_start(
                out=t[:, half * 4 : (half + 1) * 4, :],
                in_=ext[b][:, half * 4 * L : (half + 1) * 4 * L],
            )

        # ---- weight loads (sync queue), interleaved with the first inputs
        # so the first projection group's dependencies land ASAP ----
        ident = consts.tile([128, 128], bf, tag="ident")

        # ---- per-batch input tiles + loads (scalar queue) ----
        xin = {}

        # shared tags: the bufs=1 pool reuses the same buffer for batch 1;
        # Tile's WAR tracking makes each b1 load wait for b0's last reader.
        # Loads are issued lazily (right after the point in the program where
        # the WAR dependency clears) so a pending b1 load never head-of-line
        # blocks the ScalarE queue in front of exp work.
        def load_one(b, which):
            d = xin.setdefault(b, {})
            if which in ("qt", "kt", "vt", "imt"):
                ext = {"qt": qt, "kt": kt, "vt": vt, "imt": imt}[which]
                t = inp.tile([128, 8, L], bf, tag=which, name=which + "_sb")
                for half in range(2):
                    load_x_half(t, ext, b, half)
            elif which == "aug":
                t = inp.tile([128, 4, H * AUGW], bf, tag="aug", name="aug_sb")
                nc.sync.dma_start(out=t, in_=aug[b])
            else:
                ext = {"kp1": kp1, "kp2": kp2}[which]
                t = inp.tile([128, 4, L], bf, tag=which, name=which + "_sb")
                nc.sync.dma_start(out=t, in_=ext[b])
            d[which] = t

        # startup-critical order: wq/qt in fine-grained interleaved chunks so
        # the first projection matmuls can start after ~400KB instead of 3MB
        xin[0] = {}
        xin[0]["qt"] = inp.tile([128, 8, L], bf, tag="qt", name="qt_sb")
        wq_t = consts.tile([128, 8, HS], bf, tag="wq", name="wq_sb")
        w_sb["wq"] = wq_t
        for c in range(4):
            nc.sync.dma_start(
                out=wq_t[:, 2 * c : 2 * c + 2, :],
                in_=wq[:, 2 * c * HS : (2 * c + 2) * HS],
            )
            nc.sync.dma_start(
                out=xin[0]["qt"][:, 2 * c : 2 * c + 2, :],
                in_=qt[0][:, 2 * c * L : (2 * c + 2) * L],
            )
        xin[0]["kt"] = inp.tile([128, 8, L], bf, tag="kt", name="kt_sb")
        load_weight_half("wk", wk, 0)
        load_x_half(xin[0]["kt"], kt, 0, 0)
        load_weight_half("wk", wk, 1)
        load_x_half(xin[0]["kt"], kt, 0, 1)
        for half in range(2):
            load_weight_half("wv", wv, half)
        nc.sync.dma_start(out=ident, in_=idt[:, :])
        for which in ("vt", "imt", "aug", "kp1", "kp2"):
            load_one(0, which)
        for half in range(2):
            load_weight_half("wm", wm, half)

        # ---- per-batch working tiles ----
        st = {}
        for b in range(BPC):
            st[b] = {
                "qh": proj.tile([128, 8, L], bf, tag="qh", name="qh_sb"),
                "kh": proj.tile([128, 8, L], bf, tag="kh", name="kh_sb"),
                "vaug": proj.tile(
                    [128, 4, H * AUGW], bf, tag="vaug", name="vaug_sb"
                ),
                "att": proj.tile([128, 8, L], bf, tag="att", name="att_sb"),
            }

        # ---- projection groups (one PSUM group each; interleavable thunks) ----
        def proj_qk_group(b, wname, dstname, ot):
            wt = w_sb[wname]
            xsb = xin[b]["qt" if wname == "wq" else "kt"]
            dst = st[b][dstname]
            ps = psA.tile([128, 512], f32, tag="psA")
            for kc in range(8):
                nc.tensor.matmul(
                    ps,
                    wt[:, kc, ot * 128 : (ot + 1) * 128],
                    xsb[:, kc, :],
                    start=(kc == 0),
                    stop=(kc == 7),
                )
            nc.vector.tensor_copy(out=dst[:, ot, :], in_=ps)

        def vaug_group(b, jt, oh):
            vaug_sb = st[b]["vaug"]
            vt_sb = xin[b]["vt"]
            if oh == 0:
                nc.vector.memset(
                    vaug_sb[:, jt, :].rearrange("p (h x) -> p h x", x=AUGW)[:, :, 64],
                    1.0,
                )
            ps = psA.tile([128, 512], f32, tag="psA")
            for kc in range(8):
                nc.tensor.matmul(
                    ps,
                    vt_sb[:, kc, jt * 128 : (jt + 1) * 128],
                    w_sb["wv"][:, kc, oh * 512 : (oh + 1) * 512],
                    start=(kc == 0),
                    stop=(kc == 7),
                )
            dst_ap = vaug_sb[
                :, jt, oh * 8 * AUGW : (oh + 1) * 8 * AUGW
            ].rearrange("p (h x) -> p h x", x=AUGW)[:, :, 0:64]
            nc.vector.tensor_copy(
                out=dst_ap, in_=ps.rearrange("p (h x) -> p h x", x=64)
            )

        def outproj_group(b, it, oh):
            att_sb = st[b]["att"]
            ps = psA.tile([128, 512], f32, tag="psA")
            for kc in range(8):
                nc.tensor.matmul(
                    ps,
                    att_sb[:, kc, it * 128 : (it + 1) * 128],
                    w_sb["wm"][:, kc, oh * 512 : (oh + 1) * 512],
                    start=(kc == 0),
                    stop=(kc == 7),
                )
            ob = evac.tile([128, 512], bf, tag="ob")
            nc.vector.tensor_copy(out=ob, in_=ps)
            # batch 1's stores ride the (by-then idle) ScalarE HWDGE queue so
            # the kernel tail doesn't serialize behind sync's transposes
            eng = nc.scalar if b == 1 else nc.sync
            eng.dma_start(
                out=out[b, it * 128 : (it + 1) * 128, oh * 512 : (oh + 1) * 512],
                in_=ob,
            )

        # ---- attention stages ----
        def score_stage(b, hp, lhs_sb, rhs_fn, etile):
            """s^T [j,i] for both heads of pair hp + exp into etile.

            The two heads' matmuls use lhsT base partitions 0 / 64, so they
            run concurrently on the two row-halves of the PE array (outputs
            land in different PSUM banks)."""
            heads = (2 * hp, 2 * hp + 1)
            for jt in range(4):
                ps = psA.tile([128, 1024], f32, tag="psA")
                for g, h in enumerate(heads):
                    nc.tensor.matmul(
                        ps[:, g * 512 : (g + 1) * 512],
                        lhs_sb[
                            (h % 2) * 64 : (h % 2) * 64 + 64,
                            h // 2,
                            jt * 128 : (jt + 1) * 128,
                        ],
                        rhs_fn(g, h),
                        start=True,
                        stop=True,
                    )
                nc.scalar.activation(
                    out=etile[:, jt],
                    in_=ps.rearrange("p (g x) -> p g x", x=512),
                    func=Exp,
                    scale=SCALE,
                )

        def mask_stage(b, hp, kp_sb, etile):
            # in-place mask multiply, one fused op per jt covering both heads
            # (mask row broadcast across the head dim via a stride-0 AP).
            # Most tiles on DVE (bf16 2x rate); one jt per pair on GpSimd to
            # offload DVE.
            # GpSimd offloads DVE on the odd pair of each staggered duo: its
            # ~2.2us op latency hides behind the even pair's pv matmuls
            # instead of gating its own pair's accumulation.
            for jt in range(4):
                kpb = kp_sb[:, jt, :].unsqueeze(1).broadcast_to([128, 2, L])
                eng = nc.gpsimd if (jt == 3 and hp % 2 == 1) else nc.vector
                eng.tensor_mul(etile[:, jt], etile[:, jt], kpb)

        def pv_stage(b, hp, emtile, rhs_sb):
            """pv natural [i, 4*65] per head -> normalized dl pair [128,4,128].

            One fused broadcast-multiply per head turns the raw PSUM pv tile
            into the normalized bf16 dl tile (recip row-sums broadcast along
            d via a stride-0 AP)."""
            heads = (2 * hp, 2 * hp + 1)
            dl = small.tile([128, 4, 128], bf, tag="dl")
            for g, h in enumerate(heads):
                pspv = psT.tile([128, 4, AUGW], f32, tag="tail")
                for it in range(4):
                    for jt in range(4):
                        nc.tensor.matmul(
                            pspv[:, it, :],
                            emtile[:, jt, g, it * 128 : (it + 1) * 128],
                            rhs_sb[:, jt, h * AUGW : (h + 1) * AUGW],
                            start=(jt == 0),
                            stop=(jt == 3),
                        )
                r1 = small.tile([128, 4], f32, tag="r1")
                nc.vector.reciprocal(r1, pspv[:, :, 64])
                nc.vector.tensor_mul(
                    dl[:, :, g * 64 : (g + 1) * 64],
                    pspv[:, :, 0:64],
                    r1.unsqueeze(-1).broadcast_to([128, 4, 64]),
                )
            return dl

        def mod_stage(b, hp, emtile):
            """round-1 tail: pv + normalize + PE-transpose + add qh -> qn_pair.

            This transpose is on the s2 latency chain, so it stays on TensorE
            (275ns) instead of the 1.2us-per-op DMA xbar path."""
            dl = pv_stage(b, hp, emtile, xin[b]["aug"])
            pst = psT.tile([128, 512], bf, tag="tail", name="pst")
            for it in range(4):
                nc.tensor.transpose(
                    pst[:, it * 128 : (it + 1) * 128], dl[:, it, :], ident
                )
            qn_pair = small.tile([128, 512], bf, tag="qnp")
            nc.vector.tensor_add(qn_pair, pst, st[b]["qh"][:, hp, :])
            return qn_pair

        def av_stage(b, hp, emtile):
            """round-2 tail: av + normalize + DMA-transpose -> att[:, hp, :]."""
            dl = pv_stage(b, hp, emtile, st[b]["vaug"])
            for it in range(4):
                nc.sync.dma_start_transpose(
                    out=st[b]["att"][:, hp, it * 128 : (it + 1) * 128],
                    in_=dl[:, it, :],
                )

        def s1_stage(b, hp):
            e1 = ework.tile([128, 4, 2, L], bf, tag="e")
            qh_sb = st[b]["qh"]
            score_stage(
                b,
                hp,
                xin[b]["imt"],
                lambda g, h: qh_sb[(h % 2) * 64 : (h % 2) * 64 + 64, h // 2, :],
                e1,
            )
            mask_stage(b, hp, xin[b]["kp1"], e1)
            return e1

        def s2_stage(b, hp, qn_pair):
            e2 = ework.tile([128, 4, 2, L], bf, tag="e")
            score_stage(
                b,
                hp,
                st[b]["kh"],
                lambda g, h: qn_pair[(h % 2) * 64 : (h % 2) * 64 + 64, :],
                e2,
            )
            mask_stage(b, hp, xin[b]["kp2"], e2)
            return e2

        def attention(b, fillers, hooks=None, fill_from=0):
            """two-pair software pipeline; after each stage, issue one filler
            thunk (other batch's projection / output-projection group) to keep
            the PE stream dense. hooks[(stage, pair)] thunks run right after
            that stage (used to issue next-batch input loads at the points
            where their WAR dependencies have just cleared). fill_from delays
            filler consumption to later slots (e.g. to cover the pipeline
            drain at the end of the last batch's attention)."""
            hooks = hooks or {}
            slot = [0]

            def post(stage, p):
                for t in hooks.get((stage, p), ()):
                    t()
                if fillers and slot[0] >= fill_from:
                    fillers.pop(0)()
                slot[0] += 1

            for base in range(0, 8, 2):
                p0, p1 = base, base + 1
                e1_0 = s1_stage(b, p0)
                post("s1", p0)
                e1_1 = s1_stage(b, p1)
                post("s1", p1)
                qn_0 = mod_stage(b, p0, e1_0)
                post("mod", p0)
                qn_1 = mod_stage(b, p1, e1_1)
                post("mod", p1)
                e2_0 = s2_stage(b, p0, qn_0)
                post("s2", p0)
                e2_1 = s2_stage(b, p1, qn_1)
                post("s2", p1)
                av_stage(b, p0, e2_0)
                post("av", p0)
                av_stage(b, p1, e2_1)
                post("av", p1)
            while fillers:
                fillers.pop(0)()

        # ---- schedule ----
        # batch 0 projections up front (dense PE work while inputs land);
        # each b1 bulk load is issued right after its b0 reader finishes.
        for wname, dstname in (("wq", "qh"), ("wk", "kh")):
            for ot in range(8):
                proj_qk_group(0, wname, dstname, ot)
            load_one(1, "qt" if wname == "wq" else "kt")
        for jt in range(4):
            for oh in range(2):
                vaug_group(0, jt, oh)
        load_one(1, "vt")

        # batch 0 attention, filled with batch 1 projections
        b1_proj = []
        for wname, dstname in (("wq", "qh"), ("wk", "kh")):
            for ot in range(8):
                b1_proj.append(
                    lambda wname=wname, dstname=dstname, ot=ot: proj_qk_group(
                        1, wname, dstname, ot
                    )
                )
        for jt in range(4):
            for oh in range(2):
                b1_proj.append(lambda jt=jt, oh=oh: vaug_group(1, jt, oh))
        hooks0 = {
            ("s1", 7): [lambda: load_one(1, "imt"), lambda: load_one(1, "kp1")],
            ("mod", 7): [lambda: load_one(1, "aug")],
            ("s2", 7): [lambda: load_one(1, "kp2")],
        }
        attention(0, b1_proj, hooks0)

        # batch 1 attention; batch 0's output projection fills the BACK half
        # of the pipeline so dense PE work covers the attention drain
        b0_out = []
        for it in range(4):
            for oh in range(2):
                b0_out.append(lambda it=it, oh=oh: outproj_group(0, it, oh))
        attention(1, b0_out, fill_from=22)

        # batch 1 output projection (tail)
        for it in range(4):
            for oh in range(2):
                outproj_group(1, it, oh)

    nc.compile()
    return nc


def _get_nc():
    if "nc" not in _CACHE:
        _CACHE["nc"] = _build_nc()
    return _CACHE["nc"]


def _prep_inputs(v, k, q, img_abs, Wv, Wk, Wq, Wm, abs_mask, mask):
    import ml_dtypes

    bf16 = ml_dtypes.bfloat16
    f32 = np.float32

    def swz(x, nt):  # [B, nt*128, F] -> [B, 128, nt*F] partition-contiguous
        b, r, f = x.shape
        return np.ascontiguousarray(
            x.reshape(b, nt, 128, f).transpose(0, 2, 1, 3).reshape(b, 128, nt * f)
        )

    def t_bf(x):  # [B, L, HS] -> [B, 128, 8*L] bf16 swizzled
        xt = np.swapaxes(np.asarray(x, f32), 1, 2)
        return swz(xt, 8).astype(bf16)

    qt = t_bf(q)
    ktr = t_bf(k)
    vtr = t_bf(v)
    imt = t_bf(img_abs)

    img = np.asarray(img_abs, f32)
    augf = np.empty((B, L, H * AUGW), f32)
    augf.reshape(B, L, H, AUGW)[..., :64] = img.reshape(B, L, H, 64)
    augf.reshape(B, L, H, AUGW)[..., 64] = 1.0
    augv = swz(augf, 4).astype(bf16)

    def keepT(m):  # [B, 1, L, L] bool -> (1-m)^T swizzled bf16
        kf = 1.0 - np.asarray(m, f32)[:, 0]
        return swz(np.swapaxes(kf, 1, 2), 4).astype(bf16)

    kp1 = keepT(abs_mask)
    kp2 = keepT(mask)

    def wT(w):
        wt = np.asarray(w, f32).T  # [i, o]
        return swz(wt[None], 8)[0].astype(bf16)

    wqs, wks, wvs, wms = wT(Wq), wT(Wk), wT(Wv), wT(Wm)
    ident = np.eye(128, dtype=bf16)

    in_maps = []
    for c in range(NCORES):
        s = slice(c * BPC, (c + 1) * BPC)
        in_maps.append(
            {
                "qt": qt[s],
                "kt": ktr[s],
                "vt": vtr[s],
                "imt": imt[s],
                "aug": augv[s],
                "kp1": kp1[s],
                "kp2": kp2[s],
                "wq": wqs,
                "wk": wks,
                "wv": wvs,
                "wm": wms,
                "idt": ident,
            }
        )
    return in_maps


def kernel(v, k, q, img_abs, Wv, Wk, Wq, Wm, abs_mask, mask, _trace=False):
    _ensure_concourse()
    from concourse.bass_utils import run_bass_kernel_spmd

    in_maps = _prep_inputs(v, k, q, img_abs, Wv, Wk, Wq, Wm, abs_mask, mask)
    nc = _get_nc()
    res = run_bass_kernel_spmd(nc, in_maps, core_ids=list(range(NCORES)), trace=_trace)
    outp = np.concatenate([res.results[i]["out"] for i in range(NCORES)], axis=0)
    outp = np.asarray(outp, np.float32)  # device stores bf16; upcast on host
    if _trace:
        _CACHE["last_result"] = res
    return outp


# revision 38
# speedup vs baseline: 1.0044x; 1.0044x over previous
"""Trainium2 Bass kernel for nn_ABS_MHAtt (masked two-round multi-head attention).

Strategy: pure data-parallel over batch (B=16 -> 2 batches per NeuronCore, 8 cores,
no collectives). Host-side preprocessing (inside kernel()) pre-transposes
activations/weights into the [contraction, free] layouts the TensorEngine wants and
pre-converts everything to bf16, so the device kernel does zero layout conversion.

Per-core device kernel (per batch):
  - qhT/khT projections in transposed form [o, i]; v projected in natural form [j, o]
    directly into an "augmented" layout with a ones column per head (the ones column
    makes the PV/AV matmul also produce the softmax row-sum).
  - Per head: scores computed transposed [j, i] (contraction over d=64, head pairs
    row-tiled onto the two PE array halves), exp on ScalarE, masking by multiplying
    with (1-mask)^T (split across VectorE and GpSimdE), PV/AV with E as the
    stationary operand, one fused broadcast-multiply normalize per head, and DMA
    xbar transposes (not TensorE) to repack [i, d] tiles back to [d, i].

v2 scheduling: the two batches are software-pipelined at the core level — batch 1's
projection matmul groups are interleaved into batch 0's attention phase, and batch
0's output projection into batch 1's attention phase, so the PE never idles long
enough for the HAM clock gate to re-throttle. Input DMAs ride the ScalarE HWDGE
queue; weights, transposes and output stores ride the Sync queue, keeping the
latency-critical xbar transposes out of the bulk-load FIFO.
"""

import os
import sys

import numpy as np


def _ensure_concourse():
    try:
        import concourse.bass  # noqa: F401
        return
    except Exception:
        pass
    for p in ("/opt/trn_rl_repo", "/root/.axon_site/_ro/trn_rl_repo"):
        if os.path.isdir(p) and p not in sys.path:
            sys.path.insert(0, p)
            try:
                import concourse.bass  # noqa: F401
                return
            except Exception:
                sys.path.remove(p)
    raise ImportError("cannot import concourse (bass)")


B, L, HS = 16, 512, 1024
H, D = 16, 64
NCORES = 8
BPC = B // NCORES  # batches per core
SCALE = 1.0 / 8.0  # 1/sqrt(D)
AUGW = 65  # per-head augmented width (D + ones column)

_CACHE = {}


def _build_nc():
    _ensure_concourse()
    import concourse.bass as bass  # noqa: F401
    import concourse.mybir as mybir
    import concourse.tile as tile
    from concourse import bacc
    from contextlib import ExitStack

    bf = mybir.dt.bfloat16
    f32 = mybir.dt.float32
    Exp = mybir.ActivationFunctionType.Exp

    nc = bacc.Bacc()

    # all inputs host-preswizzled to [128, free] per-partition-contiguous
    # layouts so every load is one cheap 2D DMA
    qt = nc.declare_dram_parameter("qt", [BPC, 128, 8 * L], bf, isOutput=False)
    kt = nc.declare_dram_parameter("kt", [BPC, 128, 8 * L], bf, isOutput=False)
    vt = nc.declare_dram_parameter("vt", [BPC, 128, 8 * L], bf, isOutput=False)
    imt = nc.declare_dram_parameter("imt", [BPC, 128, 8 * L], bf, isOutput=False)
    aug = nc.declare_dram_parameter(
        "aug", [BPC, 128, 4 * H * AUGW], bf, isOutput=False
    )
    kp1 = nc.declare_dram_parameter("kp1", [BPC, 128, 4 * L], bf, isOutput=False)
    kp2 = nc.declare_dram_parameter("kp2", [BPC, 128, 4 * L], bf, isOutput=False)
    wq = nc.declare_dram_parameter("wq", [128, 8 * HS], bf, isOutput=False)
    wk = nc.declare_dram_parameter("wk", [128, 8 * HS], bf, isOutput=False)
    wv = nc.declare_dram_parameter("wv", [128, 8 * HS], bf, isOutput=False)
    wm = nc.declare_dram_parameter("wm", [128, 8 * HS], bf, isOutput=False)
    idt = nc.declare_dram_parameter("idt", [128, 128], bf, isOutput=False)
    out = nc.declare_dram_parameter("out", [BPC, L, HS], bf, isOutput=True)

    with ExitStack() as ctx:
        tc = ctx.enter_context(tile.TileContext(nc))
        consts = ctx.enter_context(tc.tile_pool(name="consts", bufs=1))
        inp = ctx.enter_context(tc.tile_pool(name="inp", bufs=1))
        proj = ctx.enter_context(tc.tile_pool(name="proj", bufs=2))
        ework = ctx.enter_context(tc.tile_pool(name="ework", bufs=2))
        small = ctx.enter_context(tc.tile_pool(name="small", bufs=3))
        evac = ctx.enter_context(tc.tile_pool(name="evac", bufs=2))
        psA = ctx.enter_context(tc.tile_pool(name="psA", bufs=3, space="PSUM"))
        psT = ctx.enter_context(tc.tile_pool(name="psT", bufs=2, space="PSUM"))

        w_sb = {}

        def load_weight_half(name, wext, half):
            if half == 0 and name not in w_sb:
                t = consts.tile([128, 8, HS], bf, tag=name, name=name + "_sb")
                w_sb[name] = t
            t = w_sb[name]
            nc.sync.dma_start(
                out=t[:, half * 4 : (half + 1) * 4, :],
                in_=wext[:, half * 4 * HS : (half + 1) * 4 * HS],
            )

        def load_x_half(t, ext, b, half):
            nc.sync.dma_start(
                out=t[:, half * 4 : (half + 1) * 4, :],
                in_=ext[b][:, half * 4 * L : (half + 1) * 4 * L],
            )

        # ---- weight loads (sync queue), interleaved with the first inputs
        # so the first projection group's dependencies land ASAP ----
        ident = consts.tile([128, 128], bf, tag="ident")

        # ---- per-batch input tiles + loads (scalar queue) ----
        xin = {}

        # shared tags: the bufs=1 pool reuses the same buffer for batch 1;
        # Tile's WAR tracking makes each b1 load wait for b0's last reader.
        # Loads are issued lazily (right after the point in the program where
        # the WAR dependency clears) so a pending b1 load never head-of-line
        # blocks the ScalarE queue in front of exp work.
        def load_one(b, which):
            d = xin.setdefault(b, {})
            if which in ("qt", "kt", "vt", "imt"):
                ext = {"qt": qt, "kt": kt, "vt": vt, "imt": imt}[which]
                t = inp.tile([128, 8, L], bf, tag=which, name=which + "_sb")
                for half in range(2):
                    load_x_half(t, ext, b, half)
            elif which == "aug":
                t = inp.tile([128, 4, H * AUGW], bf, tag="aug", name="aug_sb")
                nc.sync.dma_start(out=t, in_=aug[b])
            else:
                ext = {"kp1": kp1, "kp2": kp2}[which]
                t = inp.tile([128, 4, L], bf, tag=which, name=which + "_sb")
                nc.sync.dma_start(out=t, in_=ext[b])
            d[which] = t

        # startup-critical order: wq/qt in fine-grained interleaved chunks so
        # the first projection matmuls can start after ~400KB instead of 3MB
        xin[0] = {}
        xin[0]["qt"] = inp.tile([128, 8, L], bf, tag="qt", name="qt_sb")
        wq_t = consts.tile([128, 8, HS], bf, tag="wq", name="wq_sb")
        w_sb["wq"] = wq_t
        for c in range(4):
            nc.sync.dma_start(
                out=wq_t[:, 2 * c : 2 * c + 2, :],
                in_=wq[:, 2 * c * HS : (2 * c + 2) * HS],
            )
            nc.sync.dma_start(
                out=xin[0]["qt"][:, 2 * c : 2 * c + 2, :],
                in_=qt[0][:, 2 * c * L : (2 * c + 2) * L],
            )
        xin[0]["kt"] = inp.tile([128, 8, L], bf, tag="kt", name="kt_sb")
        load_weight_half("wk", wk, 0)
        load_x_half(xin[0]["kt"], kt, 0, 0)
        load_weight_half("wk", wk, 1)
        load_x_half(xin[0]["kt"], kt, 0, 1)
        for half in range(2):
            load_weight_half("wv", wv, half)
        nc.sync.dma_start(out=ident, in_=idt[:, :])
        for which in ("vt", "imt", "aug", "kp1", "kp2"):
            load_one(0, which)
        for half in range(2):
            load_weight_half("wm", wm, half)

        # ---- per-batch working tiles ----
        st = {}
        for b in range(BPC):
            st[b] = {
                "qh": proj.tile([128, 8, L], bf, tag="qh", name="qh_sb"),
                "kh": proj.tile([128, 8, L], bf, tag="kh", name="kh_sb"),
                "vaug": proj.tile(
                    [128, 4, H * AUGW], bf, tag="vaug", name="vaug_sb"
                ),
                "att": proj.tile([128, 8, L], bf, tag="att", name="att_sb"),
            }

        # ---- projection groups (one PSUM group each; interleavable thunks) ----
        def proj_qk_part(b, wname, dstname, ot, part, state):
            """half of a projection PSUM group; part 0 opens the psum tile,
            part 1 finishes the accumulation and evacuates. Split so filler
            thunks are ~0.9us of PE work for tighter gap coverage."""
            wt = w_sb[wname]
            xsb = xin[b]["qt" if wname == "wq" else "kt"]
            dst = st[b][dstname]
            if part == 0:
                state["ps"] = psA.tile([128, 512], f32, tag="psA", name="ps_pj")
            ps = state["ps"]
            for kc in range(4 * part, 4 * part + 4):
                nc.tensor.matmul(
                    ps,
                    wt[:, kc, ot * 128 : (ot + 1) * 128],
                    xsb[:, kc, :],
                    start=(kc == 0),
                    stop=(kc == 7),
                )
            if part == 1:
                if dstname == "qh":
                    nc.scalar.copy(out=dst[:, ot, :], in_=ps)
                else:
                    nc.vector.tensor_copy(out=dst[:, ot, :], in_=ps)

        def proj_qk_group(b, wname, dstname, ot):
            state = {}
            proj_qk_part(b, wname, dstname, ot, 0, state)
            proj_qk_part(b, wname, dstname, ot, 1, state)

        def vaug_group(b, jt, oh):
            vaug_sb = st[b]["vaug"]
            vt_sb = xin[b]["vt"]
            if oh == 0:
                nc.vector.memset(
                    vaug_sb[:, jt, :].rearrange("p (h x) -> p h x", x=AUGW)[:, :, 64],
                    1.0,
                )
            ps = psA.tile([128, 512], f32, tag="psA")
            for kc in range(8):
                nc.tensor.matmul(
                    ps,
                    vt_sb[:, kc, jt * 128 : (jt + 1) * 128],
                    w_sb["wv"][:, kc, oh * 512 : (oh + 1) * 512],
                    start=(kc == 0),
                    stop=(kc == 7),
                )
            dst_ap = vaug_sb[
                :, jt, oh * 8 * AUGW : (oh + 1) * 8 * AUGW
            ].rearrange("p (h x) -> p h x", x=AUGW)[:, :, 0:64]
            nc.vector.tensor_copy(
                out=dst_ap, in_=ps.rearrange("p (h x) -> p h x", x=64)
            )

        def outproj_group(b, it, oh):
            att_sb = st[b]["att"]
            ps = psA.tile([128, 512], f32, tag="psA")
            for kc in range(8):
                nc.tensor.matmul(
                    ps,
                    att_sb[:, kc, it * 128 : (it + 1) * 128],
                    w_sb["wm"][:, kc, oh * 512 : (oh + 1) * 512],
                    start=(kc == 0),
                    stop=(kc == 7),
                )
            ob = evac.tile([128, 512], bf, tag="ob")
            nc.vector.tensor_copy(out=ob, in_=ps)
            # batch 1's stores ride the (by-then idle) ScalarE HWDGE queue so
            # the kernel tail doesn't serialize behind sync's transposes
            eng = nc.scalar if b == 1 else nc.sync
            eng.dma_start(
                out=out[b, it * 128 : (it + 1) * 128, oh * 512 : (oh + 1) * 512],
                in_=ob,
            )

        # ---- attention stages ----
        def score_stage(b, hp, lhs_sb, rhs_fn, etile):
            """s^T [j,i] for both heads of pair hp + exp into etile.

            The two heads' matmuls use lhsT base partitions 0 / 64, so they
            run concurrently on the two row-halves of the PE array (outputs
            land in different PSUM banks)."""
            heads = (2 * hp, 2 * hp + 1)
            for jt in range(4):
                ps = psA.tile([128, 1024], f32, tag="psA")
                for g, h in enumerate(heads):
                    nc.tensor.matmul(
                        ps[:, g * 512 : (g + 1) * 512],
                        lhs_sb[
                            (h % 2) * 64 : (h % 2) * 64 + 64,
                            h // 2,
                            jt * 128 : (jt + 1) * 128,
                        ],
                        rhs_fn(g, h),
                        start=True,
                        stop=True,
                    )
                nc.scalar.activation(
                    out=etile[:, jt],
                    in_=ps.rearrange("p (g x) -> p g x", x=512),
                    func=Exp,
                    scale=SCALE,
                )

        def mask_stage(b, hp, kp_sb, etile):
            # in-place mask multiply, one fused op per jt covering both heads
            # (mask row broadcast across the head dim via a stride-0 AP).
            # Most tiles on DVE (bf16 2x rate); one jt per pair on GpSimd to
            # offload DVE.
            # GpSimd offloads DVE on the odd pair of each staggered duo: its
            # ~2.2us op latency hides behind the even pair's pv matmuls
            # instead of gating its own pair's accumulation.
            for jt in range(4):
                kpb = kp_sb[:, jt, :].unsqueeze(1).broadcast_to([128, 2, L])
                eng = nc.gpsimd if (jt == 3 and hp % 2 == 1) else nc.vector
                eng.tensor_mul(etile[:, jt], etile[:, jt], kpb)

        def pv_stage(b, hp, emtile, rhs_sb):
            """pv natural [i, 4*65] per head -> normalized dl pair [128,4,128].

            One fused broadcast-multiply per head turns the raw PSUM pv tile
            into the normalized bf16 dl tile (recip row-sums broadcast along
            d via a stride-0 AP)."""
            heads = (2 * hp, 2 * hp + 1)
            dl = small.tile([128, 4, 128], bf, tag="dl")
            for g, h in enumerate(heads):
                pspv = psT.tile([128, 4, AUGW], f32, tag="tail")
                for it in range(4):
                    for jt in range(4):
                        nc.tensor.matmul(
                            pspv[:, it, :],
                            emtile[:, jt, g, it * 128 : (it + 1) * 128],
                            rhs_sb[:, jt, h * AUGW : (h + 1) * AUGW],
                            start=(jt == 0),
                            stop=(jt == 3),
                        )
                r1 = small.tile([128, 4], f32, tag="r1")
                nc.vector.reciprocal(r1, pspv[:, :, 64])
                nc.vector.tensor_mul(
                    dl[:, :, g * 64 : (g + 1) * 64],
                    pspv[:, :, 0:64],
                    r1.unsqueeze(-1).broadcast_to([128, 4, 64]),
                )
            return dl

        def mod_stage(b, hp, emtile):
            """round-1 tail: pv + normalize + PE-transpose + add qh -> qn_pair.

            This transpose is on the s2 latency chain, so it stays on TensorE
            (275ns) instead of the 1.2us-per-op DMA xbar path."""
            dl = pv_stage(b, hp, emtile, xin[b]["aug"])
            pst = psT.tile([128, 512], bf, tag="tail", name="pst")
            for it in range(4):
                nc.tensor.transpose(
                    pst[:, it * 128 : (it + 1) * 128], dl[:, it, :], ident
                )
            qn_pair = small.tile([128, 512], bf, tag="qnp")
            nc.vector.tensor_add(qn_pair, pst, st[b]["qh"][:, hp, :])
            return qn_pair

        def av_stage(b, hp, emtile):
            """round-2 tail: av + normalize + DMA-transpose -> att[:, hp, :]."""
            dl = pv_stage(b, hp, emtile, st[b]["vaug"])
            for it in range(4):
                nc.sync.dma_start_transpose(
                    out=st[b]["att"][:, hp, it * 128 : (it + 1) * 128],
                    in_=dl[:, it, :],
                )

        def s1_stage(b, hp):
            e1 = ework.tile([128, 4, 2, L], bf, tag="e")
            qh_sb = st[b]["qh"]
            score_stage(
                b,
                hp,
                xin[b]["imt"],
                lambda g, h: qh_sb[(h % 2) * 64 : (h % 2) * 64 + 64, h // 2, :],
                e1,
            )
            mask_stage(b, hp, xin[b]["kp1"], e1)
            return e1

        def s2_stage(b, hp, qn_pair):
            e2 = ework.tile([128, 4, 2, L], bf, tag="e")
            score_stage(
                b,
                hp,
                st[b]["kh"],
                lambda g, h: qn_pair[(h % 2) * 64 : (h % 2) * 64 + 64, :],
                e2,
            )
            mask_stage(b, hp, xin[b]["kp2"], e2)
            return e2

        def attention(b, fillers, hooks=None, fill_from=0):
            """two-pair software pipeline; after each stage, issue one filler
            thunk (other batch's projection / output-projection group) to keep
            the PE stream dense. hooks[(stage, pair)] thunks run right after
            that stage (used to issue next-batch input loads at the points
            where their WAR dependencies have just cleared). fill_from delays
            filler consumption to later slots (e.g. to cover the pipeline
            drain at the end of the last batch's attention)."""
            hooks = hooks or {}
            slot = [0]

            def post(stage, p):
                for t in hooks.get((stage, p), ()):
                    t()
                if fillers and slot[0] >= fill_from:
                    fillers.pop(0)()
                slot[0] += 1

            for base in range(0, 8, 2):
                p0, p1 = base, base + 1
                e1_0 = s1_stage(b, p0)
                post("s1", p0)
                e1_1 = s1_stage(b, p1)
                post("s1", p1)
                qn_0 = mod_stage(b, p0, e1_0)
                post("mod", p0)
                qn_1 = mod_stage(b, p1, e1_1)
                post("mod", p1)
                e2_0 = s2_stage(b, p0, qn_0)
                post("s2", p0)
                e2_1 = s2_stage(b, p1, qn_1)
                post("s2", p1)
                av_stage(b, p0, e2_0)
                post("av", p0)
                av_stage(b, p1, e2_1)
                post("av", p1)
            while fillers:
                fillers.pop(0)()

        # ---- schedule ----
        # batch 0 projections up front (dense PE work while inputs land);
        # each b1 bulk load is issued right after its b0 reader finishes.
        for wname, dstname in (("wq", "qh"), ("wk", "kh")):
            for ot in range(8):
                proj_qk_group(0, wname, dstname, ot)
            load_one(1, "qt" if wname == "wq" else "kt")
        for jt in range(4):
            for oh in range(2):
                vaug_group(0, jt, oh)
        load_one(1, "vt")

        # batch 0 attention, filled with batch 1 projections (half-group
        # granularity so each filler is ~0.9us of PE work)
        b1_proj = []
        for wname, dstname in (("wq", "qh"), ("wk", "kh")):
            for ot in range(8):
                state = {}
                for part in range(2):
                    b1_proj.append(
                        lambda wname=wname, dstname=dstname, ot=ot, part=part, state=state: proj_qk_part(
                            1, wname, dstname, ot, part, state
                        )
                    )
        for jt in range(4):
            for oh in range(2):
                b1_proj.append(lambda jt=jt, oh=oh: vaug_group(1, jt, oh))
        hooks0 = {
            ("s1", 7): [lambda: load_one(1, "imt"), lambda: load_one(1, "kp1")],
            ("mod", 7): [lambda: load_one(1, "aug")],
            ("s2", 7): [lambda: load_one(1, "kp2")],
        }
        attention(0, b1_proj, hooks0)

        # batch 1 attention; batch 0's output projection fills the BACK half
        # of the pipeline so dense PE work covers the attention drain
        b0_out = []
        for it in range(4):
            for oh in range(2):
                b0_out.append(lambda it=it, oh=oh: outproj_group(0, it, oh))
        attention(1, b0_out, fill_from=22)

        # batch 1 output projection (tail)
        for it in range(4):
            for oh in range(2):
                outproj_group(1, it, oh)

    nc.compile()
    return nc


def _get_nc():
    if "nc" not in _CACHE:
        _CACHE["nc"] = _build_nc()
    return _CACHE["nc"]


def _prep_inputs(v, k, q, img_abs, Wv, Wk, Wq, Wm, abs_mask, mask):
    import ml_dtypes

    bf16 = ml_dtypes.bfloat16
    f32 = np.float32

    def swz(x, nt):  # [B, nt*128, F] -> [B, 128, nt*F] partition-contiguous
        b, r, f = x.shape
        return np.ascontiguousarray(
            x.reshape(b, nt, 128, f).transpose(0, 2, 1, 3).reshape(b, 128, nt * f)
        )

    def t_bf(x):  # [B, L, HS] -> [B, 128, 8*L] bf16 swizzled
        xt = np.swapaxes(np.asarray(x, f32), 1, 2)
        return swz(xt, 8).astype(bf16)

    qt = t_bf(q)
    ktr = t_bf(k)
    vtr = t_bf(v)
    imt = t_bf(img_abs)

    img = np.asarray(img_abs, f32)
    augf = np.empty((B, L, H * AUGW), f32)
    augf.reshape(B, L, H, AUGW)[..., :64] = img.reshape(B, L, H, 64)
    augf.reshape(B, L, H, AUGW)[..., 64] = 1.0
    augv = swz(augf, 4).astype(bf16)

    def keepT(m):  # [B, 1, L, L] bool -> (1-m)^T swizzled bf16
        kf = 1.0 - np.asarray(m, f32)[:, 0]
        return swz(np.swapaxes(kf, 1, 2), 4).astype(bf16)

    kp1 = keepT(abs_mask)
    kp2 = keepT(mask)

    def wT(w):
        wt = np.asarray(w, f32).T  # [i, o]
        return swz(wt[None], 8)[0].astype(bf16)

    wqs, wks, wvs, wms = wT(Wq), wT(Wk), wT(Wv), wT(Wm)
    ident = np.eye(128, dtype=bf16)

    in_maps = []
    for c in range(NCORES):
        s = slice(c * BPC, (c + 1) * BPC)
        in_maps.append(
            {
                "qt": qt[s],
                "kt": ktr[s],
                "vt": vtr[s],
                "imt": imt[s],
                "aug": augv[s],
                "kp1": kp1[s],
                "kp2": kp2[s],
                "wq": wqs,
                "wk": wks,
                "wv": wvs,
                "wm": wms,
                "idt": ident,
            }
        )
    return in_maps


def kernel(v, k, q, img_abs, Wv, Wk, Wq, Wm, abs_mask, mask, _trace=False):
    _ensure_concourse()
    from concourse.bass_utils import run_bass_kernel_spmd

    in_maps = _prep_inputs(v, k, q, img_abs, Wv, Wk, Wq, Wm, abs_mask, mask)
    nc = _get_nc()
    res = run_bass_kernel_spmd(nc, in_maps, core_ids=list(range(NCORES)), trace=_trace)
    outp = np.concatenate([res.results[i]["out"] for i in range(NCORES)], axis=0)
    outp = np.asarray(outp, np.float32)  # device stores bf16; upcast on host
    if _trace:
        _CACHE["last_result"] = res
    return outp
